# revision 22
# baseline (speedup 1.0000x reference)
"""Trainium2 Bass kernel for nn_NodeFeatures (GNN message passing).

Math (per batch b):
    Ux  = (x @ U_w.T + U_b) * 0.5                      # (N, H)
    Vx  = (x @ V_w.T + V_b) * 0.5                      # (N, H)
    agg[i,h]   = sum_j gate[i,j,h] * Vx[j,h]
    denom[i,h] = 1e-20 + sum_j gate[i,j,h]
    out = Ux + agg / denom

Sharding: data-parallel over batch B=8 across the 8 NeuronCores (one batch
per core); H x H weights replicated.

Per-core plan (memory-bound; all DMA transfers serialize on the modeled
DMA engines, and transfer time charges OUTPUT bytes, x2 when descriptors
are <512B):
  - gate is DMA'd f32->fp8e4m3 (SWDGE cast) as [j=128, i_chunk, h] tiles:
    ~46.6us stream (vs ~93us for bf16/f32 — the 128-elem DRAM h-runs force
    the small-descriptor penalty, so only the 1-byte dtype halves time).
    e4m3 quantization lands ~2.5e-3 rel on the output (gate only feeds the
    agg/denom ratio, which is ~7% of output magnitude).  Early DMAs are
    row-split (21/21/22, 32/32) so the first desc-gen — which gates the
    serialized stream — is minimal while later gens still outpace
    transfers; ident/vxo Pool setup is slotted where gen has enough lead.
  - For each h, gate is the stationary lhsT [j, i] (strided AP) and
    rhs [j, 2] = [Vx[:,h], 1] computes agg AND denom together into PSUM
    [i, (h, 2)].  Chunks at PSUM partition base 0 fuse both j-halves into
    one DoubleRow-fp8 matmul (k-tile dim); other bases (ISA restriction)
    accumulate the two j-halves via start/stop pairs.  Matmul cost ~ rhs
    free size (2 cols) and weight loads pipeline away, so PE is far off
    the critical path; the old DVE gate*Vx multiply, PSUM drains, and
    output repack disappear entirely.  The stream is ordered so the LAST
    chunk is a base-0 (DoubleRow) one, halving the post-stream PE burst.
  - U_b/V_b are loaded as [1,H] rows and broadcast into the Ux/Vx PSUM
    group via a K=1 ones-row matmul (no [128,H] broadcast DMAs).
  - Epilogue per 128-node block: rec = 1/denom, res = agg*rec + Ux, one
    contiguous DMA out (block 0 mid-stream, block 1 in the tail).
Timeline: ~3.6us head (first desc-gen) + ~46.8us stream + ~5.6us tail
(DMA sem + 128 DoubleRow matmuls + epilogue + out DMA + exit drains).
"""

import sys

import numpy as np

try:
    import concourse.bass as bass  # noqa: F401
except ImportError:  # pragma: no cover
    sys.path.insert(0, "/opt/trn_rl_repo")

from contextlib import ExitStack

import concourse.bacc as bacc
import concourse.mybir as mybir
import concourse.tile as tile
from concourse import bass_utils
from concourse.masks import make_identity

F32 = mybir.dt.float32
FP8 = mybir.dt.float8e4

B, N, H = 8, 256, 128
NCORES = 8
JT = 2                     # j-halves (contraction tiles)
# i-chunks as (node_start, size): chunk boundaries must land on PSUM
# tile_position-legal offsets.  The HW ISA only allows DoubleRow matmuls at
# dst partition base 0, so the stream is ordered to make the LAST chunk a
# base-0 one (block 1's [128:192]) — its post-stream PE burst then runs in
# DoubleRow.  Early chunks' DMAs are row-split so the first desc-gen (which
# gates the whole serialized DMA stream) is as short as possible while
# later gens (994 + 43.5ns/row) still outpace transfers (91ns/row).
CHUNKS = [(0, 64), (64, 64), (192, 64), (128, 64)]
ROW_SPLITS = {0: (21, 21, 22), 64: (28, 36), 192: (32, 32), 128: (64,)}
SANDWICH_AFTER = 8  # DMA count after which the Pool setup is emitted
assert sorted(s for s, _ in CHUNKS) == [0, 64, 128, 192]
assert sum(ic for _, ic in CHUNKS) == N
assert all(sum(ROW_SPLITS[s]) == ic for s, ic in CHUNKS)


def build_program():
    """Build the per-core Bass program (identical on all 8 cores)."""
    # Bass.__init__ memsets four const tiles (0.0 / 1.0 / bf16-1.0 / u8-127)
    # on the Pool queue before any user instruction; none are read by this
    # program (BIR verifier confirms no-reader), yet their Q7 launches delay
    # the first gate desc-gen, which gates the whole serialized DMA stream.
    # Skip them during construction (memset resolves via
    # BassEitherVectorEngine, not BassSharedVectorInterface).
    eng_cls = bass.BassEitherVectorEngine
    orig_memset = eng_cls.memset

    def _memset_skip_const(self, ap, constant):
        t = getattr(ap, "tensor", None)
        if t is not None and t.name.startswith("const-"):
            return None
        return orig_memset(self, ap, constant)

    eng_cls.memset = _memset_skip_const
    try:
        nc = bacc.Bacc("TRN2", target_bir_lowering=False, debug=False,
                       num_devices=NCORES)
    finally:
        eng_cls.memset = orig_memset

    x_d = nc.dram_tensor("x", [N, H], F32, kind="ExternalInput").ap()
    g_d = nc.dram_tensor("gate", [N, N, H], F32, kind="ExternalInput").ap()
    uw_d = nc.dram_tensor("U_w", [H, H], F32, kind="ExternalInput").ap()
    ub_d = nc.dram_tensor("U_b", [H], F32, kind="ExternalInput").ap()
    vw_d = nc.dram_tensor("V_w", [H, H], F32, kind="ExternalInput").ap()
    vb_d = nc.dram_tensor("V_b", [H], F32, kind="ExternalInput").ap()
    out_d = nc.dram_tensor("out", [N, H], F32, kind="ExternalOutput").ap()

    # DRAM gate view [jt, j, i, h] (i sliced per chunk below)
    gjv = g_d.rearrange("i (t j) h -> t j i h", j=128)

    with tile.TileContext(nc) as tc, ExitStack() as ctx:
        const = ctx.enter_context(tc.tile_pool(name="const", bufs=1))

        # ---- main stream over gate: issue DMAs before everything ---------
        # Desc-gen (Pool engine) for chunk 0 starts at t~0 so the first
        # transfer hits the (serialized) DMA engines as early as possible.
        # The Pool-engine setup (identity + vxo memset) is sandwiched after
        # chunk 0's desc-gen: chunk 0 gen has ~1.3us of slack before its
        # transfer, later gens (0.34ns/desc) outpace transfers (0.71ns/desc),
        # and setup consumers (transposes -> Vx -> matmuls) are ready long
        # before chunk 0's tiles land.
        # One buffer per chunk (all resident, ~64KB/partition total): no
        # ring reuse, so no gate DMA ever waits on downstream compute.
        gate_pool = ctx.enter_context(tc.tile_pool(name="gate", bufs=1))
        ident = const.tile([128, 128], F32)
        vxo = const.tile([128, JT, 2, H], FP8)
        ones_row = const.tile([1, 128], F32)
        gts = []
        ndma = 0
        for s, ic in CHUNKS:
            gt = gate_pool.tile([128, JT, ic, H], FP8, tag=f"g{s}",
                                name=f"gt_{s}")
            for t in range(JT):
                r0 = 0
                for r in ROW_SPLITS[s]:
                    nc.gpsimd.dma_start(gt[:, t, r0:r0 + r, :],
                                        gjv[t, :, s + r0:s + r0 + r, :])
                    r0 += r
                    ndma += 1
                    if ndma == SANDWICH_AFTER:
                        # Slot the Pool setup where desc-gen has built up
                        # enough lead over the transfers that it causes no
                        # stream bubble; the matmuls (which need vxo) all
                        # still finish well before the stream does.
                        make_identity(nc, ident)
                        # vxo[j,t,c,h]: c=0 -> Vx fp8, c=1 -> 1.0
                        nc.gpsimd.memset(vxo[:, :, 1, :], 1.0)
                        nc.gpsimd.memset(ones_row, 1.0)
            gts.append((gt, s, ic))

        # ---- small input loads (SP + ACT HWDGE queues; transfers slot in
        # ahead of the first gate transfer, which waits on desc-gen) --------
        x_sb = const.tile([128, 2, H], F32)           # [i_in_block, blk, h]
        nc.sync.dma_start(x_sb, x_d.rearrange("(b i) h -> i b h", i=128))
        uw_sb = const.tile([H, H], F32)
        nc.sync.dma_start(uw_sb, uw_d)
        # vw rides the ACT HWDGE queue: the SP queue's third HWDGE setup
        # would finish ~20ns after the first gate transfer starts, pushing
        # vw's transfer mid-stream; on ACT it lands in the head idle window.
        vw_sb = const.tile([H, H], F32)
        nc.scalar.dma_start(vw_sb, vw_d)
        # biases as single-row tiles (~1ns transfers); broadcast to all
        # partitions happens inside the Ux/Vx PSUM group via a K=1 ones-row
        # matmul, so no [128,H] broadcast DMA is needed.
        ub_sb = const.tile([1, H], F32)
        nc.sync.dma_start(ub_sb, ub_d[None, :])
        vb_sb = const.tile([1, H], F32)
        nc.sync.dma_start(vb_sb, vb_d[None, :])

        # ---- setup: transposes, Ux, Vx->vxo ------------------------------
        # All copies on DVE: an ACT instruction would trigger a 1.3us
        # activation-table load that delays the ACT HWDGE bias DMAs.
        xT = const.tile([H, N], F32)                  # [h, n]
        uwT = const.tile([H, H], F32)                 # [h, k] -> [k, h]
        vwT = const.tile([H, H], F32)
        ux_sb = const.tile([128, 2, H], F32)          # [i_in_block, blk, h]

        with tc.tile_pool(name="spsum", bufs=2, space="PSUM") as spsum:
            for blk in range(2):
                pt = spsum.tile([128, 128], F32, tag="tr")
                nc.tensor.transpose(pt, x_sb[:, blk, :], ident)
                nc.vector.tensor_copy(xT[:, blk * 128:(blk + 1) * 128], pt)
            ptu = spsum.tile([128, 128], F32, tag="tr")
            nc.tensor.transpose(ptu, uw_sb, ident)
            nc.vector.tensor_copy(uwT, ptu)
            ptv = spsum.tile([128, 128], F32, tag="tr")
            nc.tensor.transpose(ptv, vw_sb, ident)
            nc.vector.tensor_copy(vwT, ptv)

            for blk in range(2):
                lhs = xT[:, blk * 128:(blk + 1) * 128]
                pv = spsum.tile([128, 128], F32, tag="mm")
                nc.tensor.matmul(pv, lhsT=lhs, rhs=vwT, start=True, stop=False)
                nc.tensor.matmul(pv, lhsT=ones_row, rhs=vb_sb,
                                 start=False, stop=True)
                # vx = (x@Vw.T + V_b)*0.5, cast to fp8 on write
                nc.vector.tensor_scalar_mul(vxo[:, blk, 0, :], pv, 0.5)
                pu = spsum.tile([128, 128], F32, tag="mm")
                nc.tensor.matmul(pu, lhsT=lhs, rhs=uwT, start=True, stop=False)
                nc.tensor.matmul(pu, lhsT=ones_row, rhs=ub_sb,
                                 start=False, stop=True)
                nc.vector.tensor_scalar_mul(ux_sb[:, blk, :], pu, 0.5)

        # ---- per-h matmuls: agg|denom into PSUM [i, (h, 2)] --------------
        mpsum = ctx.enter_context(tc.tile_pool(name="mpsum", bufs=2,
                                               space="PSUM"))
        psums = [mpsum.tile([128, H, 2], F32, tag="AD", name=f"ps{b}")
                 for b in range(2)]

        def epilogue(blk):
            # (TensorTensor divide does not lower in the HW compiler; use
            # reciprocal + multiply, as the iterative-divide DVE op does.)
            p = psums[blk]
            rec = const.tile([128, H], F32, name=f"rec{blk}")
            nc.vector.reciprocal(rec, p[:, :, 1])
            res = const.tile([128, H], F32, name=f"res{blk}")
            nc.vector.tensor_mul(res, p[:, :, 0], rec)
            nc.vector.tensor_add(res, res, ux_sb[:, blk, :])
            nc.sync.dma_start(
                out_d.rearrange("(b i) h -> i b h", i=128)[:, blk, :], res)

        # DoubleRow fp8: both 128-row j-halves (k-tiles) in one instruction —
        # lhsT [j, t, i], rhs [j, t, 2].  One start&stop group per h, so the
        # PSUM zero-region constraint is trivially met, and the PE burst
        # after the last gate tile is halved.  The HW ISA rejects DoubleRow
        # with nonzero dst partition base (s3d3_mm_valid_dst_partition), so
        # those chunks fall back to per-j-half start/stop pairs.
        remaining = [128, 128]
        for gt, s, ic in gts:
            blk, pbase = divmod(s, 128)
            p = psums[blk]
            for h in range(H):
                if pbase == 0:
                    nc.tensor.matmul(
                        p[pbase:pbase + ic, h, :],
                        lhsT=gt[:, :, :, h],
                        rhs=vxo[:, :, :, h],
                        start=True, stop=True,
                        perf_mode=mybir.MatmulPerfMode.DoubleRow)
                else:
                    for t in range(JT):
                        nc.tensor.matmul(
                            p[pbase:pbase + ic, h, :],
                            lhsT=gt[:, t, :, h],
                            rhs=vxo[:, t, :, h],
                            start=(t == 0), stop=(t == JT - 1))
            remaining[blk] -= ic
            if remaining[blk] == 0:
                epilogue(blk)

    nc.compile()
    return nc


_NC_CACHE = None


def _get_program():
    global _NC_CACHE
    if _NC_CACHE is None:
        _NC_CACHE = build_program()
    return _NC_CACHE


def kernel(**inputs: np.ndarray) -> np.ndarray:
    x = np.ascontiguousarray(np.asarray(inputs["x"], dtype=np.float32))
    gate = np.ascontiguousarray(
        np.asarray(inputs["edge_gate"], dtype=np.float32))
    u_w = np.ascontiguousarray(np.asarray(inputs["U_w"], dtype=np.float32))
    u_b = np.ascontiguousarray(np.asarray(inputs["U_b"], dtype=np.float32))
    v_w = np.ascontiguousarray(np.asarray(inputs["V_w"], dtype=np.float32))
    v_b = np.ascontiguousarray(np.asarray(inputs["V_b"], dtype=np.float32))

    nc = _get_program()
    in_maps = [
        {
            "x": x[c],
            "gate": gate[c],
            "U_w": u_w,
            "U_b": u_b,
            "V_w": v_w,
            "V_b": v_b,
        }
        for c in range(NCORES)
    ]
    res = bass_utils.run_bass_kernel_spmd(
        nc, in_maps, core_ids=list(range(NCORES)))
    return np.stack([res.results[c]["out"] for c in range(NCORES)], axis=0)
